# revision 4
# baseline (speedup 1.0000x reference)
"""CartesianMACE rank-0 fused kernel for 8 trn2 NeuronCores (v7).

Only the rank-0 path reaches the output (ranks 1/2 of the reference are
dead code), so per node n with 16x16 mats A=cw0[0,n], B=mw0[0,n],
D=cw1[0,n], E=mw1[0,n] and 16-vecs x=h0[n], m0=msg0_r0[n], m1=msg1_r0[n]:

    s[n] = colsum(D) . (A x + B m0) + colsum(E) . m1
    out  = [sum_n s[n] w_pred[0,n], sum_n s[n] w_pred[1,n]] + b_pred

v6 (76.1us) was DVE-bound: 55us of TENSOR_TENSOR at the 2x-mode rate
(~0.68ns/col), DMA engines only 54% occupied, Act/PE/GpSimd idle.
TensorTensor tops out at 2x; the fold-add count is conserved under
associativity, so v7 moves the D/E colsum tree off the DVE entirely:

  * DE is laid out on host as 4 j-row slabs [q, t, m, g, k, jq]; the
    gpsimd SWDGE queue lands slab 0 then accumulates slabs 1..3 onto the
    same tile (dma accum_op=add), so the 16->4 colsum fold happens in
    the DMA engines (which have ~46% spare capacity).  DVE keeps only
    the 4->2 pair fold + the epilogue pair collapse for the de side:
    -18.8k DVE cols (~-12us).
  * ab span0 is split into m-halves (two transfers, two muls) so the
    first mul starts ~2us earlier; everything else as v6: bf16 2x DVE
    fold trees, pairs surviving into the epilogue, nodes on partitions,
    one merged SP-queue transfer per span.
  * Per-core (128,2) partials summed on host with b_pred (the head's
    all-reduce over the node dim).
"""

import sys

for _p in ("/opt/trn_rl_repo", "/root/.axon_site/_ro/trn_rl_repo"):
    if _p not in sys.path:
        sys.path.append(_p)

import numpy as np
import ml_dtypes

BF16 = ml_dtypes.bfloat16

N, CH = 50000, 16
CORES = 8
T, S = 7, 7          # supertiles per core, groups per supertile
GP = T * S           # 49 groups of 128 nodes per core
NP = CORES * T * 128 * S  # 50176 padded nodes
SPANS = [(0, 1), (1, 2), (3, 2), (5, 2)]  # (first supertile, length)

_cache = {}
TRACE = False


def _split_multiwait(nc, mybir):
    """This walrus build accepts a single sync-wait per instruction, but Tile
    attaches one wait per producer proc. Split: keep the last wait on the
    instruction and hoist the rest onto fresh same-engine Drain carriers
    inserted immediately before it (engines execute their stream in-order,
    so semantics are identical)."""
    for fn in nc.m.functions:
        for bb in fn.blocks:
            insts = bb.instructions  # live list
            i = 0
            while i < len(insts):
                ins = insts[i]
                si = ins.sync_info
                if si is not None and len(si.on_wait) > 1:
                    waits = list(si.on_wait)
                    ins.sync_info = mybir.SyncInfo(
                        on_wait=waits[-1:], on_update=list(si.on_update))
                    for k, w in enumerate(waits[:-1]):
                        insts.insert(i + k, mybir.InstDrain(
                            name=f"{ins.name}_w{k}", opcode="Drain",
                            engine=ins.engine, ins=[], outs=[],
                            sync_info=mybir.SyncInfo(on_wait=[w], on_update=[]),
                        ))
                    i += len(waits) - 1
                i += 1


def _build_nc():
    import concourse.bass as bass
    import concourse.tile as tile
    import concourse.mybir as mybir

    f32 = mybir.dt.float32
    b16 = mybir.dt.bfloat16
    P = 128

    nc = bass.Bass("TRN2", target_bir_lowering=False, debug=False,
                   num_devices=CORES)

    # xm rides at the head of the ab tensor: one transfer (xm+ab span0
    # m-half) gates the first mul instead of two serialized completions
    ab_d = nc.dram_tensor("ab", [P, T * 224 + T * 3584], b16,
                          kind="ExternalInput").ap()
    # de: 4 j-row slabs [q, t, m, g, k, jq]; slab q holds rows 4q+jq
    de_d = nc.dram_tensor("de", [P, T * 3584], b16, kind="ExternalInput").ap()
    m1_d = nc.dram_tensor("m1", [P, T * 112], b16, kind="ExternalInput").ap()
    w_d = nc.dram_tensor("w", [P, 2 * GP], b16, kind="ExternalInput").ap()
    o_d = nc.dram_tensor("o", [P, 2], f32, kind="ExternalOutput").ap()

    F2R = 2 * T * 224  # 3136: [t, m, g, k, r2] per-partition layout
    F1 = T * 224       # 1568
    QS = T * 896       # de slab stride (one j-quarter of all supertiles)

    with tile.TileContext(nc) as tc:
        with (
            tc.tile_pool(name="big", bufs=1) as big,
            tc.tile_pool(name="work", bufs=1) as work,
        ):
            ab_all = big.tile([P, T * 224 + T * 3584], b16)
            xm_sb = ab_all[:, 0:T * 224]
            XO = T * 224            # ab data offset within ab_all
            w_sb = big.tile([P, 2 * GP], b16)
            # ct2[:, 0:3136] = cd pairs (D|E colsums), [:, 3136:] = t pairs
            ct2 = big.tile([P, 2 * F2R], b16)
            vv = big.tile([P, F1], b16)     # [t, sel, g, k]: tn | m1
            cdf = big.tile([P, F1], b16)    # [t, m, g, k]: cd | ce
            pr = big.tile([P, F1], b16)
            tn2 = big.tile([P, F1], b16)    # m-summed t pairs

            # ---- all DMAs upfront; SP carries ab/m1/w, SWDGE carries de ----
            # span0 ab split into m-halves for a faster DVE ramp
            nc.sync.dma_start(out=ab_all[:, 0:XO + 1792],
                              in_=ab_d[:, 0:XO + 1792])
            nc.sync.dma_start(out=ab_all[:, XO + 1792:XO + 3584],
                              in_=ab_d[:, XO + 1792:XO + 3584])
            for t0, ts in SPANS[1:]:
                E0, EN = t0 * 3584, ts * 3584
                nc.sync.dma_start(out=ab_all[:, XO + E0:XO + E0 + EN],
                                  in_=ab_d[:, XO + E0:XO + E0 + EN])
            # de: per span, land slab0 then accumulate slabs 1..3 (the
            # 16->4 colsum fold runs in the DMA engines)
            de4s = []
            for t0, ts in SPANS:
                de4 = work.tile([P, 2 * 896], b16, tag="de4", bufs=4)
                de4s.append(de4)
                for q in range(4):
                    src = de_d[:, q * QS + t0 * 896:q * QS + (t0 + ts) * 896]
                    if q == 0:
                        nc.gpsimd.dma_start(out=de4[:, 0:ts * 896], in_=src)
                    else:
                        nc.gpsimd.dma_start(out=de4[:, 0:ts * 896], in_=src,
                                            accum_op=mybir.AluOpType.add)
            # epilogue-only inputs last: m1 into its vv slot (sel=1), w
            nc.sync.dma_start(
                out=vv[:, :].rearrange("p (t s x) -> p t s x",
                                       t=T, s=2, x=112)[:, :, 1],
                in_=m1_d.rearrange("p (t x) -> p t x", t=T, x=112))
            nc.sync.dma_start(out=w_sb[:, :], in_=w_d)

            t4 = work.tile([P, 2 * 896], b16)
            for si, (t0, ts) in enumerate(SPANS):
                de4 = de4s[si]
                EN = ts * 3584
                U = ts * 14          # (t, m, g) groups in span
                tmp = work.tile([P, 2 * 3584], b16, tag="tmp", bufs=2)
                t8 = work.tile([P, 2 * 1792], b16, tag="t8", bufs=2)
                gjk = lambda ap, u: ap.rearrange("p (u j k) -> p u j k",
                                                 u=u, j=16, k=16)
                # products: span0 in m-halves (each gated by its own
                # transfer), later spans in one fat op
                if si == 0:
                    for h in range(2):
                        xm_h = (xm_sb[:, h * 112:(h + 1) * 112]
                                .rearrange("p (u k) -> p u k", u=7, k=16)
                                .unsqueeze(2).broadcast_to((P, 7, 16, 16)))
                        nc.vector.tensor_mul(
                            out=gjk(tmp[:, h * 1792:(h + 1) * 1792], 7),
                            in0=gjk(ab_all[:, XO + h * 1792:XO + (h + 1) * 1792], 7),
                            in1=xm_h)
                else:
                    xm_bc = (xm_sb[:, t0 * 224:(t0 + ts) * 224]
                             .rearrange("p (u k) -> p u k", u=U, k=16)
                             .unsqueeze(2).broadcast_to((P, U, 16, 16)))
                    nc.vector.tensor_mul(out=gjk(tmp[:, 0:EN], U),
                                         in0=gjk(ab_all[:, XO + t0 * 3584:
                                                        XO + (t0 + ts) * 3584], U),
                                         in1=xm_bc)
                # ab fold 16->8
                HV = ts * 224        # 16-wide rows in span
                a16 = tmp[:, 0:EN].rearrange("p (v k) -> p v k", v=HV, k=16)
                e8a = t8[:, 0:HV * 8].rearrange("p (v k) -> p v k",
                                                v=HV, k=8)
                nc.vector.tensor_add(out=e8a, in0=a16[:, :, 0:8],
                                     in1=a16[:, :, 8:16])
                # ab 8->4
                e4 = t4[:, 0:HV * 4].rearrange("p (v k) -> p v k", v=HV, k=4)
                nc.vector.tensor_add(out=e4, in0=e8a[:, :, 0:4],
                                     in1=e8a[:, :, 4:8])
                # ab 4->2, pairs kept -> t segment of ct2
                t_v = (ct2[:, F2R + t0 * 448:F2R + (t0 + ts) * 448]
                       .rearrange("p (y r) -> p y r", y=HV, r=2))
                nc.vector.tensor_add(out=t_v, in0=e4[:, :, 0:2],
                                     in1=e4[:, :, 2:4])
                # de 4->2 (DMA already folded 16->4), pairs kept -> cd seg
                d4 = de4[:, 0:ts * 896].rearrange("p (z f) -> p z f",
                                                  z=HV, f=4)
                cd_v = (ct2[:, t0 * 448:(t0 + ts) * 448]
                        .rearrange("p (z r) -> p z r", z=HV, r=2))
                nc.vector.tensor_add(out=cd_v, in0=d4[:, :, 0:2],
                                     in1=d4[:, :, 2:4])

            # ---- epilogue ----
            cd2 = ct2[:, 0:F2R]                  # [t, m, g, k, r] pairs
            t12 = ct2[:, F2R:2 * F2R]
            tmx = lambda ap: ap.rearrange("p (t m x) -> p t m x",
                                          t=T, m=2, x=224)
            # tn2[t,g,k,r] = t12[t,0,..] + t12[t,1,..]   (m-sum, 2x)
            tn2h = tn2[:, 0:F1].rearrange("p (t x) -> p t x", t=T, x=224)
            nc.vector.tensor_add(out=tn2h, in0=tmx(t12)[:, :, 0],
                                 in1=tmx(t12)[:, :, 1])
            # collapse pairs (1x, small): tn -> vv[sel=0]; cd2 -> cdf
            tr = tn2[:, 0:F1].rearrange("p (v r) -> p v r", v=F1 // 2, r=2)
            nc.vector.tensor_add(
                out=vv[:, :].rearrange("p (t s x) -> p t s x",
                                       t=T, s=2, x=112)[:, :, 0],
                in0=tr[:, :, 0].rearrange("p (t x) -> p t x", t=T, x=112),
                in1=tr[:, :, 1].rearrange("p (t x) -> p t x", t=T, x=112))
            cr = cd2.rearrange("p (v r) -> p v r", v=F2R // 2, r=2)
            nc.vector.tensor_add(out=cdf[:, :], in0=cr[:, :, 0],
                                 in1=cr[:, :, 1])
            # pr[t,m,g,k] = cdf * (tn | m1)
            nc.vector.tensor_mul(out=pr[:, :], in0=cdf[:, :], in1=vv[:, :])
            # fold [98, 16] -> [98]
            p16 = pr[:, :].rearrange("p (v k) -> p v k", v=98, k=16)
            h8 = tn2[:, 0:784].rearrange("p (v k) -> p v k", v=98, k=8)
            nc.vector.tensor_add(out=h8, in0=p16[:, :, 0:8],
                                 in1=p16[:, :, 8:16])
            h4 = tn2[:, 784:1176].rearrange("p (v k) -> p v k", v=98, k=4)
            nc.vector.tensor_add(out=h4, in0=h8[:, :, 0:4], in1=h8[:, :, 4:8])
            h2 = tn2[:, 1176:1372].rearrange("p (v k) -> p v k", v=98, k=2)
            nc.vector.tensor_add(out=h2, in0=h4[:, :, 0:2], in1=h4[:, :, 2:4])
            h1 = tn2[:, 1372:1470]
            nc.vector.tensor_add(out=h1, in0=h2[:, :, 0], in1=h2[:, :, 1])
            # s[t,g] = h1[t,0,g] + h1[t,1,g]
            s_all = tn2[:, 1470:1519]
            gm = h1.rearrange("p (t m g) -> p t m g", t=T, m=2, g=S)
            nc.vector.tensor_add(out=s_all.rearrange("p (t g) -> p t g",
                                                     t=T, g=S),
                                 in0=gm[:, :, 0], in1=gm[:, :, 1])
            # head: hm[c, tg] = s[tg] * w[c, tg]; o[c] = sum_tg hm
            hm = pr[:, 0:2 * GP].rearrange("p (c q) -> p c q", c=2, q=GP)
            nc.vector.tensor_mul(
                out=hm,
                in0=w_sb[:, :].rearrange("p (c q) -> p c q", c=2, q=GP),
                in1=s_all.unsqueeze(1).broadcast_to((P, 2, GP)))
            o_sb = big.tile([P, 2], f32)
            nc.vector.reduce_sum(out=o_sb[:, :].rearrange("p c -> p c"),
                                 in_=hm, axis=mybir.AxisListType.X)
            nc.sync.dma_start(out=o_d, in_=o_sb[:, :])

    return nc


def _get_nc():
    if "nc" not in _cache:
        _cache["nc"] = _build_nc()
        import concourse.mybir as mybir
        _split_multiwait(_cache["nc"], mybir)
    return _cache["nc"]


def kernel(h0, cw0, mw0, cw1, mw1,
           msg0_r0, msg0_r1, msg0_r2,
           msg1_r0, msg1_r1, msg1_r2,
           w_pred, b_pred):
    from concourse.bass_utils import run_bass_kernel_spmd

    nc = _get_nc()

    def pad_mat(m):
        out = np.zeros((NP, 256), np.float32)
        out[:N] = np.asarray(m, np.float32).reshape(N, 256)
        return out.reshape(CORES, T, 128, S, 16, 16)  # [c,t,p,g,j,k]

    A5 = pad_mat(cw0[0])
    B5 = pad_mat(mw0[0])
    # AB: [c,t,p, m,g,j,k] -> (c,t,p,3584)
    AB = np.ascontiguousarray(
        np.stack([A5, B5], axis=3).reshape(CORES, T, 128, 3584)
        .transpose(0, 2, 1, 3).reshape(CORES, 128, T * 3584)).astype(BF16)

    D5 = pad_mat(cw1[0])
    E5 = pad_mat(mw1[0])
    # DE: 4 j-row slabs, each [t, m, g, k, jq]: slab q holds rows 4q+jq;
    # the SWDGE accum transfers sum the slabs -> colsum fold 16->4 in-DMA
    D6 = D5.transpose(0, 1, 2, 3, 5, 4).reshape(CORES, T, 128, S, 16, 4, 4)
    E6 = E5.transpose(0, 1, 2, 3, 5, 4).reshape(CORES, T, 128, S, 16, 4, 4)
    ST = np.stack([D6, E6], axis=4)  # [c,t,p,g,m,k,q,jq]
    DE = np.ascontiguousarray(
        ST.transpose(0, 2, 6, 1, 4, 3, 5, 7)  # [c,p,q,t,m,g,k,jq]
        .reshape(CORES, 128, T * 3584)).astype(BF16)

    def pad_vec(v):
        out = np.zeros((NP, 16), np.float32)
        out[:N] = np.asarray(v, np.float32).reshape(N, 16)
        return out.reshape(CORES, T, 128, S, 16)

    X = pad_vec(np.asarray(h0, np.float32)[..., 0])
    M0 = pad_vec(np.asarray(msg0_r0, np.float32)[..., 0])
    XM = (np.stack([X, M0], axis=3).reshape(CORES, T, 128, 224)
          .transpose(0, 2, 1, 3).reshape(CORES, 128, T * 224)).astype(BF16)
    AB = np.ascontiguousarray(np.concatenate([XM, AB], axis=2))
    M1 = np.ascontiguousarray(
        pad_vec(np.asarray(msg1_r0, np.float32)[..., 0])
        .reshape(CORES, T, 128, 112)
        .transpose(0, 2, 1, 3).reshape(CORES, 128, T * 112)).astype(BF16)

    wp = np.zeros((2, NP), np.float32)
    wp[:, :N] = np.asarray(w_pred, np.float32)
    W = np.ascontiguousarray(
        wp.reshape(2, CORES, T, 128, S).transpose(1, 3, 0, 2, 4)
        .reshape(CORES, 128, 2 * GP)).astype(BF16)

    in_maps = [
        {"ab": AB[i], "de": DE[i], "m1": M1[i], "w": W[i]}
        for i in range(CORES)
    ]
    res = run_bass_kernel_spmd(nc, in_maps, list(range(CORES)), trace=TRACE)
    _cache["last_res"] = res
    partial = np.zeros(2, np.float64)
    for i in range(CORES):
        partial += res.results[i]["o"].astype(np.float64).sum(axis=0)
    out = (partial + np.asarray(b_pred, np.float64)).astype(np.float32)
    return out.reshape(1, 2)


# revision 13
# speedup vs baseline: 1.1429x; 1.1429x over previous
"""CartesianMACE rank-0 fused kernel for 8 trn2 NeuronCores (v7).

Only the rank-0 path reaches the output (ranks 1/2 of the reference are
dead code), so per node n with 16x16 mats A=cw0[0,n], B=mw0[0,n],
D=cw1[0,n], E=mw1[0,n] and 16-vecs x=h0[n], m0=msg0_r0[n], m1=msg1_r0[n]:

    s[n] = colsum(D) . (A x + B m0) + colsum(E) . m1
    out  = [sum_n s[n] w_pred[0,n], sum_n s[n] w_pred[1,n]] + b_pred

v6 (76.1us) was DVE-bound: 55us of TENSOR_TENSOR at the 2x-mode rate
(~0.68ns/col), DMA engines only 54% occupied, Act/PE/GpSimd idle.
TensorTensor tops out at 2x; the fold-add count is conserved under
associativity, so v7 moves the D/E colsum tree off the DVE entirely:

  * DE is laid out on host as 4 j-row slabs [q, t, m, g, k, jq]; the
    gpsimd SWDGE queue lands slab 0 then accumulates slabs 1..3 onto the
    same tile (dma accum_op=add), so the 16->4 colsum fold happens in
    the DMA engines (which have ~46% spare capacity).  DVE keeps only
    the 4->2 pair fold + the epilogue pair collapse for the de side:
    -18.8k DVE cols (~-12us).
  * ab span0 is split into m-halves (two transfers, two muls) so the
    first mul starts ~2us earlier; everything else as v6: bf16 2x DVE
    fold trees, pairs surviving into the epilogue, nodes on partitions,
    one merged SP-queue transfer per span.
  * Per-core (128,2) partials summed on host with b_pred (the head's
    all-reduce over the node dim).
"""

import sys

for _p in ("/opt/trn_rl_repo", "/root/.axon_site/_ro/trn_rl_repo"):
    if _p not in sys.path:
        sys.path.append(_p)

import numpy as np
import ml_dtypes

BF16 = ml_dtypes.bfloat16

N, CH = 50000, 16
CORES = 8
T, S = 7, 7          # supertiles per core, groups per supertile
GP = T * S           # 49 groups of 128 nodes per core
NP = CORES * T * 128 * S  # 50176 padded nodes
SPANS = [(0, 1), (1, 2), (3, 2), (5, 2)]  # (first supertile, length)

_cache = {}
TRACE = False


def _split_multiwait(nc, mybir):
    """This walrus build accepts a single sync-wait per instruction, but Tile
    attaches one wait per producer proc. Split: keep the last wait on the
    instruction and hoist the rest onto fresh same-engine Drain carriers
    inserted immediately before it (engines execute their stream in-order,
    so semantics are identical)."""
    for fn in nc.m.functions:
        for bb in fn.blocks:
            insts = bb.instructions  # live list
            i = 0
            while i < len(insts):
                ins = insts[i]
                si = ins.sync_info
                if si is not None and len(si.on_wait) > 1:
                    waits = list(si.on_wait)
                    ins.sync_info = mybir.SyncInfo(
                        on_wait=waits[-1:], on_update=list(si.on_update))
                    for k, w in enumerate(waits[:-1]):
                        insts.insert(i + k, mybir.InstDrain(
                            name=f"{ins.name}_w{k}", opcode="Drain",
                            engine=ins.engine, ins=[], outs=[],
                            sync_info=mybir.SyncInfo(on_wait=[w], on_update=[]),
                        ))
                    i += len(waits) - 1
                i += 1


def _build_nc():
    import concourse.bass as bass
    import concourse.tile as tile
    import concourse.mybir as mybir

    f32 = mybir.dt.float32
    b16 = mybir.dt.bfloat16
    P = 128

    nc = bass.Bass("TRN2", target_bir_lowering=False, debug=False,
                   num_devices=CORES)

    # xm rides at the head of the ab tensor: one transfer (xm+ab span0
    # m-half) gates the first mul instead of two serialized completions
    ab_d = nc.dram_tensor("ab", [P, T * 224 + T * 3584], b16,
                          kind="ExternalInput").ap()
    # de: 4 j-row slabs [q, t, m, g, k, jq]; slab q holds rows 4q+jq
    de_d = nc.dram_tensor("de", [P, T * 3584], b16, kind="ExternalInput").ap()
    m1_d = nc.dram_tensor("m1", [P, T * 112], b16, kind="ExternalInput").ap()
    w_d = nc.dram_tensor("w", [P, 2 * GP], b16, kind="ExternalInput").ap()
    o_d = nc.dram_tensor("o", [P, 2], f32, kind="ExternalOutput").ap()

    F2R = 2 * T * 224  # 3136: [t, m, g, k, r2] per-partition layout
    F1 = T * 224       # 1568
    QS = T * 896       # de slab stride (one j-quarter of all supertiles)

    with tile.TileContext(nc) as tc:
        with (
            tc.tile_pool(name="big", bufs=1) as big,
            tc.tile_pool(name="work", bufs=1) as work,
        ):
            ab_all = big.tile([P, T * 224 + T * 3584], b16)
            xm_sb = ab_all[:, 0:T * 224]
            XO = T * 224            # ab data offset within ab_all
            w_sb = big.tile([P, 2 * GP], b16)
            # ct2[:, 0:3136] = cd pairs (D|E colsums), [:, 3136:] = t pairs
            ct2 = big.tile([P, 2 * F2R], b16)
            vv = big.tile([P, F1], b16)     # [t, sel, g, k]: tn | m1
            cdf = big.tile([P, F1], b16)    # [t, m, g, k]: cd | ce
            pr = big.tile([P, F1], b16)
            tn2 = big.tile([P, F1], b16)    # m-summed t pairs

            # ---- all DMAs upfront; SP carries ab/m1/w, SWDGE carries de ----
            # span0 ab split into m-halves for a faster DVE ramp
            nc.sync.dma_start(out=ab_all[:, 0:XO + 1792],
                              in_=ab_d[:, 0:XO + 1792])
            nc.sync.dma_start(out=ab_all[:, XO + 1792:XO + 3584],
                              in_=ab_d[:, XO + 1792:XO + 3584])
            for t0, ts in SPANS[1:]:
                E0, EN = t0 * 3584, ts * 3584
                nc.sync.dma_start(out=ab_all[:, XO + E0:XO + E0 + EN],
                                  in_=ab_d[:, XO + E0:XO + E0 + EN])
            # de: four independent k-quarter chains, each 4 slab transfers
            # (write + 3 accums) -> the 16->4 colsum fold runs in the DMA
            # engines.  Per-partition runs are 3136B: accum RMW is exact
            # up to ~4KB descriptor runs but corrupts above (HW-verified:
            # 6272B runs race, 3584B/3136B are exact).  Chains round-robin
            # on the in-order Pool queue so each accum's predecessor
            # finished ~4 instructions earlier -> no engine stalls; the
            # DVE-side 4->2 folds sit at the END of the DVE stream (after
            # all spans), so accum latency gates nothing.
            KQ = T * 224            # one k-quarter of all supertiles
            dq0 = big.tile([P, KQ], b16)
            dq1 = big.tile([P, KQ], b16)
            dq2 = big.tile([P, KQ], b16)
            dq3 = big.tile([P, KQ], b16)
            deq = [dq0, dq1, dq2, dq3]
            for q in range(4):
                for kq in range(4):
                    src = de_d[:, q * QS + kq * KQ:q * QS + (kq + 1) * KQ]
                    if q == 0:
                        nc.gpsimd.dma_start(out=deq[kq][:, :], in_=src)
                    else:
                        nc.gpsimd.dma_start(out=deq[kq][:, :], in_=src,
                                            accum_op=mybir.AluOpType.add)
            # epilogue-only inputs last: m1 into its vv slot (sel=1), w
            nc.sync.dma_start(
                out=vv[:, :].rearrange("p (t s x) -> p t s x",
                                       t=T, s=2, x=112)[:, :, 1],
                in_=m1_d.rearrange("p (t x) -> p t x", t=T, x=112))
            nc.sync.dma_start(out=w_sb[:, :], in_=w_d)

            t4 = work.tile([P, 2 * 896], b16)
            for si, (t0, ts) in enumerate(SPANS):
                EN = ts * 3584
                U = ts * 14          # (t, m, g) groups in span
                tmp = work.tile([P, 2 * 3584], b16, tag="tmp", bufs=2)
                t8 = work.tile([P, 2 * 1792], b16, tag="t8", bufs=2)
                gjk = lambda ap, u: ap.rearrange("p (u j k) -> p u j k",
                                                 u=u, j=16, k=16)
                # products: span0 in m-halves (each gated by its own
                # transfer), later spans in one fat op
                if si == 0:
                    for h in range(2):
                        xm_h = (xm_sb[:, h * 112:(h + 1) * 112]
                                .rearrange("p (u k) -> p u k", u=7, k=16)
                                .unsqueeze(2).broadcast_to((P, 7, 16, 16)))
                        nc.vector.tensor_mul(
                            out=gjk(tmp[:, h * 1792:(h + 1) * 1792], 7),
                            in0=gjk(ab_all[:, XO + h * 1792:XO + (h + 1) * 1792], 7),
                            in1=xm_h)
                else:
                    xm_bc = (xm_sb[:, t0 * 224:(t0 + ts) * 224]
                             .rearrange("p (u k) -> p u k", u=U, k=16)
                             .unsqueeze(2).broadcast_to((P, U, 16, 16)))
                    nc.vector.tensor_mul(out=gjk(tmp[:, 0:EN], U),
                                         in0=gjk(ab_all[:, XO + t0 * 3584:
                                                        XO + (t0 + ts) * 3584], U),
                                         in1=xm_bc)
                # ab fold 16->8
                HV = ts * 224        # 16-wide rows in span
                a16 = tmp[:, 0:EN].rearrange("p (v k) -> p v k", v=HV, k=16)
                e8a = t8[:, 0:HV * 8].rearrange("p (v k) -> p v k",
                                                v=HV, k=8)
                nc.vector.tensor_add(out=e8a, in0=a16[:, :, 0:8],
                                     in1=a16[:, :, 8:16])
                # ab 8->4
                e4 = t4[:, 0:HV * 4].rearrange("p (v k) -> p v k", v=HV, k=4)
                nc.vector.tensor_add(out=e4, in0=e8a[:, :, 0:4],
                                     in1=e8a[:, :, 4:8])
                # ab 4->2, pairs kept -> t segment of ct2
                t_v = (ct2[:, F2R + t0 * 448:F2R + (t0 + ts) * 448]
                       .rearrange("p (y r) -> p y r", y=HV, r=2))
                nc.vector.tensor_add(out=t_v, in0=e4[:, :, 0:2],
                                     in1=e4[:, :, 2:4])

            # de 4->2 (DMA already folded 16->4), pairs kept -> cd segs.
            # cd region layout [t, m, g, k16, r2] with k=(kq, kl): the
            # kq-quarter writes land at base kq*8 with kl at stride 2.
            for kq in range(4):
                d4 = deq[kq][:, :].rearrange("p (z f) -> p z f",
                                             z=T * 56, f=4)
                cd_v = (ct2[:, 0:F2R]
                        .rearrange("p (u x) -> p u x", u=T * 14, x=32)
                        [:, :, kq * 8:(kq + 1) * 8]
                        .rearrange("p u (kl r) -> p u kl r", kl=4, r=2))
                nc.vector.tensor_add(
                    out=cd_v,
                    in0=d4[:, :, 0:2].rearrange("p (u kl) f -> p u kl f",
                                                u=T * 14, kl=4),
                    in1=d4[:, :, 2:4].rearrange("p (u kl) f -> p u kl f",
                                                u=T * 14, kl=4))

            # ---- epilogue ----
            cd2 = ct2[:, 0:F2R]                  # [t, m, g, k, r] pairs
            t12 = ct2[:, F2R:2 * F2R]
            tmx = lambda ap: ap.rearrange("p (t m x) -> p t m x",
                                          t=T, m=2, x=224)
            # tn2[t,g,k,r] = t12[t,0,..] + t12[t,1,..]   (m-sum, 2x)
            tn2h = tn2[:, 0:F1].rearrange("p (t x) -> p t x", t=T, x=224)
            nc.vector.tensor_add(out=tn2h, in0=tmx(t12)[:, :, 0],
                                 in1=tmx(t12)[:, :, 1])
            # collapse pairs (1x, small): tn -> vv[sel=0]; cd2 -> cdf
            tr = tn2[:, 0:F1].rearrange("p (v r) -> p v r", v=F1 // 2, r=2)
            nc.vector.tensor_add(
                out=vv[:, :].rearrange("p (t s x) -> p t s x",
                                       t=T, s=2, x=112)[:, :, 0],
                in0=tr[:, :, 0].rearrange("p (t x) -> p t x", t=T, x=112),
                in1=tr[:, :, 1].rearrange("p (t x) -> p t x", t=T, x=112))
            cr = cd2.rearrange("p (v r) -> p v r", v=F2R // 2, r=2)
            nc.vector.tensor_add(out=cdf[:, :], in0=cr[:, :, 0],
                                 in1=cr[:, :, 1])
            # pr[t,m,g,k] = cdf * (tn | m1)
            nc.vector.tensor_mul(out=pr[:, :], in0=cdf[:, :], in1=vv[:, :])
            # fold [98, 16] -> [98]
            p16 = pr[:, :].rearrange("p (v k) -> p v k", v=98, k=16)
            h8 = tn2[:, 0:784].rearrange("p (v k) -> p v k", v=98, k=8)
            nc.vector.tensor_add(out=h8, in0=p16[:, :, 0:8],
                                 in1=p16[:, :, 8:16])
            h4 = tn2[:, 784:1176].rearrange("p (v k) -> p v k", v=98, k=4)
            nc.vector.tensor_add(out=h4, in0=h8[:, :, 0:4], in1=h8[:, :, 4:8])
            h2 = tn2[:, 1176:1372].rearrange("p (v k) -> p v k", v=98, k=2)
            nc.vector.tensor_add(out=h2, in0=h4[:, :, 0:2], in1=h4[:, :, 2:4])
            h1 = tn2[:, 1372:1470]
            nc.vector.tensor_add(out=h1, in0=h2[:, :, 0], in1=h2[:, :, 1])
            # s[t,g] = h1[t,0,g] + h1[t,1,g]
            s_all = tn2[:, 1470:1519]
            gm = h1.rearrange("p (t m g) -> p t m g", t=T, m=2, g=S)
            nc.vector.tensor_add(out=s_all.rearrange("p (t g) -> p t g",
                                                     t=T, g=S),
                                 in0=gm[:, :, 0], in1=gm[:, :, 1])
            # head: hm[c, tg] = s[tg] * w[c, tg]; o[c] = sum_tg hm
            hm = pr[:, 0:2 * GP].rearrange("p (c q) -> p c q", c=2, q=GP)
            nc.vector.tensor_mul(
                out=hm,
                in0=w_sb[:, :].rearrange("p (c q) -> p c q", c=2, q=GP),
                in1=s_all.unsqueeze(1).broadcast_to((P, 2, GP)))
            o_sb = big.tile([P, 2], f32)
            nc.vector.reduce_sum(out=o_sb[:, :].rearrange("p c -> p c"),
                                 in_=hm, axis=mybir.AxisListType.X)
            nc.sync.dma_start(out=o_d, in_=o_sb[:, :])

    return nc


def _get_nc():
    if "nc" not in _cache:
        _cache["nc"] = _build_nc()
        import concourse.mybir as mybir
        _split_multiwait(_cache["nc"], mybir)
    return _cache["nc"]


def kernel(h0, cw0, mw0, cw1, mw1,
           msg0_r0, msg0_r1, msg0_r2,
           msg1_r0, msg1_r1, msg1_r2,
           w_pred, b_pred):
    from concourse.bass_utils import run_bass_kernel_spmd

    nc = _get_nc()

    def pad_mat(m):
        out = np.zeros((NP, 256), np.float32)
        out[:N] = np.asarray(m, np.float32).reshape(N, 256)
        return out.reshape(CORES, T, 128, S, 16, 16)  # [c,t,p,g,j,k]

    A5 = pad_mat(cw0[0])
    B5 = pad_mat(mw0[0])
    # AB: [c,t,p, m,g,j,k] -> (c,t,p,3584)
    AB = np.ascontiguousarray(
        np.stack([A5, B5], axis=3).reshape(CORES, T, 128, 3584)
        .transpose(0, 2, 1, 3).reshape(CORES, 128, T * 3584)).astype(BF16)

    D5 = pad_mat(cw1[0])
    E5 = pad_mat(mw1[0])
    # DE: 4 j-row slabs x 4 k-quarter chains, each [t, m, g, kl, jq]:
    # slab q holds rows 4q+jq; the SWDGE accum transfers sum the slabs so
    # the colsum fold 16->4 happens in the DMA engines
    D6 = D5.transpose(0, 1, 2, 3, 5, 4).reshape(CORES, T, 128, S, 16, 4, 4)
    E6 = E5.transpose(0, 1, 2, 3, 5, 4).reshape(CORES, T, 128, S, 16, 4, 4)
    ST = (np.stack([D6, E6], axis=4)  # [c,t,p,g,m,k,q,jq]
          .reshape(CORES, T, 128, S, 2, 4, 4, 4, 4))  # k -> (kq, kl)
    DE = np.ascontiguousarray(
        ST.transpose(0, 2, 7, 5, 1, 4, 3, 6, 8)  # [c,p,q,kq,t,m,g,kl,jq]
        .reshape(CORES, 128, T * 3584)).astype(BF16)

    def pad_vec(v):
        out = np.zeros((NP, 16), np.float32)
        out[:N] = np.asarray(v, np.float32).reshape(N, 16)
        return out.reshape(CORES, T, 128, S, 16)

    X = pad_vec(np.asarray(h0, np.float32)[..., 0])
    M0 = pad_vec(np.asarray(msg0_r0, np.float32)[..., 0])
    XM = (np.stack([X, M0], axis=3).reshape(CORES, T, 128, 224)
          .transpose(0, 2, 1, 3).reshape(CORES, 128, T * 224)).astype(BF16)
    AB = np.ascontiguousarray(np.concatenate([XM, AB], axis=2))
    M1 = np.ascontiguousarray(
        pad_vec(np.asarray(msg1_r0, np.float32)[..., 0])
        .reshape(CORES, T, 128, 112)
        .transpose(0, 2, 1, 3).reshape(CORES, 128, T * 112)).astype(BF16)

    wp = np.zeros((2, NP), np.float32)
    wp[:, :N] = np.asarray(w_pred, np.float32)
    W = np.ascontiguousarray(
        wp.reshape(2, CORES, T, 128, S).transpose(1, 3, 0, 2, 4)
        .reshape(CORES, 128, 2 * GP)).astype(BF16)

    in_maps = [
        {"ab": AB[i], "de": DE[i], "m1": M1[i], "w": W[i]}
        for i in range(CORES)
    ]
    res = run_bass_kernel_spmd(nc, in_maps, list(range(CORES)), trace=TRACE)
    _cache["last_res"] = res
    partial = np.zeros(2, np.float64)
    for i in range(CORES):
        partial += res.results[i]["o"].astype(np.float64).sum(axis=0)
    out = (partial + np.asarray(b_pred, np.float64)).astype(np.float32)
    return out.reshape(1, 2)


# revision 15
# speedup vs baseline: 1.1432x; 1.0003x over previous
"""CartesianMACE rank-0 fused kernel for 8 trn2 NeuronCores (v7).

Only the rank-0 path reaches the output (ranks 1/2 of the reference are
dead code), so per node n with 16x16 mats A=cw0[0,n], B=mw0[0,n],
D=cw1[0,n], E=mw1[0,n] and 16-vecs x=h0[n], m0=msg0_r0[n], m1=msg1_r0[n]:

    s[n] = colsum(D) . (A x + B m0) + colsum(E) . m1
    out  = [sum_n s[n] w_pred[0,n], sum_n s[n] w_pred[1,n]] + b_pred

v6 (76.1us) was DVE-bound: 55us of TENSOR_TENSOR at the 2x-mode rate
(~0.68ns/col), DMA engines only 54% occupied, Act/PE/GpSimd idle.
TensorTensor tops out at 2x; the fold-add count is conserved under
associativity, so v7 moves the D/E colsum tree off the DVE entirely:

  * DE is laid out on host as 4 j-row slabs [q, t, m, g, k, jq]; the
    gpsimd SWDGE queue lands slab 0 then accumulates slabs 1..3 onto the
    same tile (dma accum_op=add), so the 16->4 colsum fold happens in
    the DMA engines (which have ~46% spare capacity).  DVE keeps only
    the 4->2 pair fold + the epilogue pair collapse for the de side:
    -18.8k DVE cols (~-12us).
  * ab span0 is split into m-halves (two transfers, two muls) so the
    first mul starts ~2us earlier; everything else as v6: bf16 2x DVE
    fold trees, pairs surviving into the epilogue, nodes on partitions,
    one merged SP-queue transfer per span.
  * Per-core (128,2) partials summed on host with b_pred (the head's
    all-reduce over the node dim).
"""

import sys

for _p in ("/opt/trn_rl_repo", "/root/.axon_site/_ro/trn_rl_repo"):
    if _p not in sys.path:
        sys.path.append(_p)

import numpy as np
import ml_dtypes

BF16 = ml_dtypes.bfloat16

N, CH = 50000, 16
CORES = 8
T, S = 7, 7          # supertiles per core, groups per supertile
GP = T * S           # 49 groups of 128 nodes per core
NP = CORES * T * 128 * S  # 50176 padded nodes
SPANS = [(0, 1), (1, 2), (3, 2), (5, 2)]  # (first supertile, length)

_cache = {}
TRACE = False


def _split_multiwait(nc, mybir):
    """This walrus build accepts a single sync-wait per instruction, but Tile
    attaches one wait per producer proc. Split: keep the last wait on the
    instruction and hoist the rest onto fresh same-engine Drain carriers
    inserted immediately before it (engines execute their stream in-order,
    so semantics are identical)."""
    for fn in nc.m.functions:
        for bb in fn.blocks:
            insts = bb.instructions  # live list
            i = 0
            while i < len(insts):
                ins = insts[i]
                si = ins.sync_info
                if si is not None and len(si.on_wait) > 1:
                    waits = list(si.on_wait)
                    ins.sync_info = mybir.SyncInfo(
                        on_wait=waits[-1:], on_update=list(si.on_update))
                    for k, w in enumerate(waits[:-1]):
                        insts.insert(i + k, mybir.InstDrain(
                            name=f"{ins.name}_w{k}", opcode="Drain",
                            engine=ins.engine, ins=[], outs=[],
                            sync_info=mybir.SyncInfo(on_wait=[w], on_update=[]),
                        ))
                    i += len(waits) - 1
                i += 1


def _build_nc():
    import concourse.bass as bass
    import concourse.tile as tile
    import concourse.mybir as mybir

    f32 = mybir.dt.float32
    b16 = mybir.dt.bfloat16
    P = 128

    nc = bass.Bass("TRN2", target_bir_lowering=False, debug=False,
                   num_devices=CORES)

    # xm rides at the head of the ab tensor: one transfer (xm+ab span0
    # m-half) gates the first mul instead of two serialized completions
    ab_d = nc.dram_tensor("ab", [P, T * 224 + T * 3584], b16,
                          kind="ExternalInput").ap()
    # de: 4 j-row slabs [q, t, m, g, k, jq]; slab q holds rows 4q+jq
    de_d = nc.dram_tensor("de", [P, T * 3584], b16, kind="ExternalInput").ap()
    m1_d = nc.dram_tensor("m1", [P, T * 112], b16, kind="ExternalInput").ap()
    w_d = nc.dram_tensor("w", [P, 2 * GP], b16, kind="ExternalInput").ap()
    o_d = nc.dram_tensor("o", [P, 2], f32, kind="ExternalOutput").ap()

    F2R = 2 * T * 224  # 3136: [t, m, g, k, r2] per-partition layout
    F1 = T * 224       # 1568
    QS = T * 896       # de slab stride (one j-quarter of all supertiles)

    with tile.TileContext(nc) as tc:
        with (
            tc.tile_pool(name="big", bufs=1) as big,
            tc.tile_pool(name="work", bufs=1) as work,
        ):
            ab_all = big.tile([P, T * 224 + T * 3584], b16)
            xm_sb = ab_all[:, 0:T * 224]
            XO = T * 224            # ab data offset within ab_all
            w_sb = big.tile([P, 2 * GP], b16)
            # ct2[:, 0:3136] = cd pairs (D|E colsums), [:, 3136:] = t pairs
            ct2 = big.tile([P, 2 * F2R], b16)
            vv = big.tile([P, F1], b16)     # [t, sel, g, k]: tn | m1
            cdf = big.tile([P, F1], b16)    # [t, m, g, k]: cd | ce
            pr = big.tile([P, F1], b16)
            tn2 = big.tile([P, F1], b16)    # m-summed t pairs

            # ---- all DMAs upfront; SP carries ab/m1/w, SWDGE carries de ----
            # span0 ab split into m-halves for a faster DVE ramp
            nc.sync.dma_start(out=ab_all[:, 0:XO + 1792],
                              in_=ab_d[:, 0:XO + 1792])
            nc.sync.dma_start(out=ab_all[:, XO + 1792:XO + 3584],
                              in_=ab_d[:, XO + 1792:XO + 3584])
            for t0, ts in SPANS[1:]:
                E0, EN = t0 * 3584, ts * 3584
                nc.sync.dma_start(out=ab_all[:, XO + E0:XO + E0 + EN],
                                  in_=ab_d[:, XO + E0:XO + E0 + EN])
            KQ = T * 224            # one k-quarter of all supertiles
            dq0 = big.tile([P, KQ], b16)
            dq1 = big.tile([P, KQ], b16)
            dq2 = big.tile([P, KQ], b16)
            dq3 = big.tile([P, KQ], b16)
            deq = [dq0, dq1, dq2, dq3]
            dscr = big.tile([P, 4], b16)
            # epilogue-only inputs last: m1 into its vv slot (sel=1), w
            nc.sync.dma_start(
                out=vv[:, :].rearrange("p (t s x) -> p t s x",
                                       t=T, s=2, x=112)[:, :, 1],
                in_=m1_d.rearrange("p (t x) -> p t x", t=T, x=112))
            nc.sync.dma_start(out=w_sb[:, :], in_=w_d)

            t4 = work.tile([P, 2 * 896], b16)
            for si, (t0, ts) in enumerate(SPANS):
                EN = ts * 3584
                U = ts * 14          # (t, m, g) groups in span
                tmp = work.tile([P, 2 * 3584], b16, tag="tmp", bufs=2)
                t8 = work.tile([P, 2 * 1792], b16, tag="t8", bufs=2)
                gjk = lambda ap, u: ap.rearrange("p (u j k) -> p u j k",
                                                 u=u, j=16, k=16)
                # products: span0 in m-halves (each gated by its own
                # transfer), later spans in one fat op
                if si == 0:
                    for h in range(2):
                        xm_h = (xm_sb[:, h * 112:(h + 1) * 112]
                                .rearrange("p (u k) -> p u k", u=7, k=16)
                                .unsqueeze(2).broadcast_to((P, 7, 16, 16)))
                        nc.vector.tensor_mul(
                            out=gjk(tmp[:, h * 1792:(h + 1) * 1792], 7),
                            in0=gjk(ab_all[:, XO + h * 1792:XO + (h + 1) * 1792], 7),
                            in1=xm_h)
                else:
                    xm_bc = (xm_sb[:, t0 * 224:(t0 + ts) * 224]
                             .rearrange("p (u k) -> p u k", u=U, k=16)
                             .unsqueeze(2).broadcast_to((P, U, 16, 16)))
                    nc.vector.tensor_mul(out=gjk(tmp[:, 0:EN], U),
                                         in0=gjk(ab_all[:, XO + t0 * 3584:
                                                        XO + (t0 + ts) * 3584], U),
                                         in1=xm_bc)
                # ab fold 16->8
                HV = ts * 224        # 16-wide rows in span
                a16 = tmp[:, 0:EN].rearrange("p (v k) -> p v k", v=HV, k=16)
                e8a = t8[:, 0:HV * 8].rearrange("p (v k) -> p v k",
                                                v=HV, k=8)
                nc.vector.tensor_add(out=e8a, in0=a16[:, :, 0:8],
                                     in1=a16[:, :, 8:16])
                # ab 8->4
                e4 = t4[:, 0:HV * 4].rearrange("p (v k) -> p v k", v=HV, k=4)
                nc.vector.tensor_add(out=e4, in0=e8a[:, :, 0:4],
                                     in1=e8a[:, :, 4:8])
                # ab 4->2, pairs kept -> t segment of ct2
                t_v = (ct2[:, F2R + t0 * 448:F2R + (t0 + ts) * 448]
                       .rearrange("p (y r) -> p y r", y=HV, r=2))
                nc.vector.tensor_add(out=t_v, in0=e4[:, :, 0:2],
                                     in1=e4[:, :, 2:4])

                if si == 1:
                    # de: four independent k-quarter chains, each 4 slab
                    # transfers (write + 3 accums) -> the 16->4 colsum
                    # fold runs in the DMA engines.  Per-partition runs
                    # are 3136B: accum RMW is exact up to ~4KB runs but
                    # corrupts above (HW-verified: 6272B races, 3584B/
                    # 3136B exact).  The dummy DVE reads below execute
                    # after span1 (~25us) and, being emitted BEFORE the
                    # transfers, gate the q0 writes (WAR): the whole de
                    # stream runs in the back half of the kernel where
                    # the DMA engines are idle, instead of starving the
                    # ab stream that feeds the DVE ramp (v9: 79us from
                    # exactly that).  Chains round-robin on the in-order
                    # Pool queue so each accum's predecessor finished ~4
                    # instructions earlier; the DVE-side folds sit after
                    # the span loop, so accum latency gates nothing.
                    for kq in range(4):
                        nc.vector.tensor_add(out=dscr[:, kq:kq + 1],
                                             in0=deq[kq][:, 0:1],
                                             in1=deq[kq][:, 0:1])
                    for q in range(4):
                        for kq in range(4):
                            src = de_d[:, q * QS + kq * KQ:
                                       q * QS + (kq + 1) * KQ]
                            if q == 0:
                                nc.gpsimd.dma_start(out=deq[kq][:, :],
                                                    in_=src)
                            else:
                                nc.gpsimd.dma_start(
                                    out=deq[kq][:, :], in_=src,
                                    accum_op=mybir.AluOpType.add)

            # de 4->2 (DMA already folded 16->4), pairs kept -> cd segs.
            # cd region layout [t, m, g, k16, r2] with k=(kq, kl): the
            # kq-quarter writes land at base kq*8 with kl at stride 2.
            for kq in range(4):
                d4 = deq[kq][:, :].rearrange("p (z f) -> p z f",
                                             z=T * 56, f=4)
                cd_v = (ct2[:, 0:F2R]
                        .rearrange("p (u x) -> p u x", u=T * 14, x=32)
                        [:, :, kq * 8:(kq + 1) * 8]
                        .rearrange("p u (kl r) -> p u kl r", kl=4, r=2))
                nc.vector.tensor_add(
                    out=cd_v,
                    in0=d4[:, :, 0:2].rearrange("p (u kl) f -> p u kl f",
                                                u=T * 14, kl=4),
                    in1=d4[:, :, 2:4].rearrange("p (u kl) f -> p u kl f",
                                                u=T * 14, kl=4))

            # ---- epilogue ----
            cd2 = ct2[:, 0:F2R]                  # [t, m, g, k, r] pairs
            t12 = ct2[:, F2R:2 * F2R]
            tmx = lambda ap: ap.rearrange("p (t m x) -> p t m x",
                                          t=T, m=2, x=224)
            # tn2[t,g,k,r] = t12[t,0,..] + t12[t,1,..]   (m-sum, 2x)
            tn2h = tn2[:, 0:F1].rearrange("p (t x) -> p t x", t=T, x=224)
            nc.vector.tensor_add(out=tn2h, in0=tmx(t12)[:, :, 0],
                                 in1=tmx(t12)[:, :, 1])
            # collapse pairs (1x, small): tn -> vv[sel=0]; cd2 -> cdf
            tr = tn2[:, 0:F1].rearrange("p (v r) -> p v r", v=F1 // 2, r=2)
            nc.vector.tensor_add(
                out=vv[:, :].rearrange("p (t s x) -> p t s x",
                                       t=T, s=2, x=112)[:, :, 0],
                in0=tr[:, :, 0].rearrange("p (t x) -> p t x", t=T, x=112),
                in1=tr[:, :, 1].rearrange("p (t x) -> p t x", t=T, x=112))
            cr = cd2.rearrange("p (v r) -> p v r", v=F2R // 2, r=2)
            nc.vector.tensor_add(out=cdf[:, :], in0=cr[:, :, 0],
                                 in1=cr[:, :, 1])
            # pr[t,m,g,k] = cdf * (tn | m1)
            nc.vector.tensor_mul(out=pr[:, :], in0=cdf[:, :], in1=vv[:, :])
            # fold [98, 16] -> [98]
            p16 = pr[:, :].rearrange("p (v k) -> p v k", v=98, k=16)
            h8 = tn2[:, 0:784].rearrange("p (v k) -> p v k", v=98, k=8)
            nc.vector.tensor_add(out=h8, in0=p16[:, :, 0:8],
                                 in1=p16[:, :, 8:16])
            h4 = tn2[:, 784:1176].rearrange("p (v k) -> p v k", v=98, k=4)
            nc.vector.tensor_add(out=h4, in0=h8[:, :, 0:4], in1=h8[:, :, 4:8])
            h2 = tn2[:, 1176:1372].rearrange("p (v k) -> p v k", v=98, k=2)
            nc.vector.tensor_add(out=h2, in0=h4[:, :, 0:2], in1=h4[:, :, 2:4])
            h1 = tn2[:, 1372:1470]
            nc.vector.tensor_add(out=h1, in0=h2[:, :, 0], in1=h2[:, :, 1])
            # s[t,g] = h1[t,0,g] + h1[t,1,g]
            s_all = tn2[:, 1470:1519]
            gm = h1.rearrange("p (t m g) -> p t m g", t=T, m=2, g=S)
            nc.vector.tensor_add(out=s_all.rearrange("p (t g) -> p t g",
                                                     t=T, g=S),
                                 in0=gm[:, :, 0], in1=gm[:, :, 1])
            # head: hm[c, tg] = s[tg] * w[c, tg]; o[c] = sum_tg hm
            hm = pr[:, 0:2 * GP].rearrange("p (c q) -> p c q", c=2, q=GP)
            nc.vector.tensor_mul(
                out=hm,
                in0=w_sb[:, :].rearrange("p (c q) -> p c q", c=2, q=GP),
                in1=s_all.unsqueeze(1).broadcast_to((P, 2, GP)))
            o_sb = big.tile([P, 2], f32)
            nc.vector.reduce_sum(out=o_sb[:, :].rearrange("p c -> p c"),
                                 in_=hm, axis=mybir.AxisListType.X)
            nc.sync.dma_start(out=o_d, in_=o_sb[:, :])

    return nc


def _get_nc():
    if "nc" not in _cache:
        _cache["nc"] = _build_nc()
        import concourse.mybir as mybir
        _split_multiwait(_cache["nc"], mybir)
    return _cache["nc"]


def kernel(h0, cw0, mw0, cw1, mw1,
           msg0_r0, msg0_r1, msg0_r2,
           msg1_r0, msg1_r1, msg1_r2,
           w_pred, b_pred):
    from concourse.bass_utils import run_bass_kernel_spmd

    nc = _get_nc()

    def pad_mat(m):
        out = np.zeros((NP, 256), np.float32)
        out[:N] = np.asarray(m, np.float32).reshape(N, 256)
        return out.reshape(CORES, T, 128, S, 16, 16)  # [c,t,p,g,j,k]

    A5 = pad_mat(cw0[0])
    B5 = pad_mat(mw0[0])
    # AB: [c,t,p, m,g,j,k] -> (c,t,p,3584)
    AB = np.ascontiguousarray(
        np.stack([A5, B5], axis=3).reshape(CORES, T, 128, 3584)
        .transpose(0, 2, 1, 3).reshape(CORES, 128, T * 3584)).astype(BF16)

    D5 = pad_mat(cw1[0])
    E5 = pad_mat(mw1[0])
    # DE: 4 j-row slabs x 4 k-quarter chains, each [t, m, g, kl, jq]:
    # slab q holds rows 4q+jq; the SWDGE accum transfers sum the slabs so
    # the colsum fold 16->4 happens in the DMA engines
    D6 = D5.transpose(0, 1, 2, 3, 5, 4).reshape(CORES, T, 128, S, 16, 4, 4)
    E6 = E5.transpose(0, 1, 2, 3, 5, 4).reshape(CORES, T, 128, S, 16, 4, 4)
    ST = (np.stack([D6, E6], axis=4)  # [c,t,p,g,m,k,q,jq]
          .reshape(CORES, T, 128, S, 2, 4, 4, 4, 4))  # k -> (kq, kl)
    DE = np.ascontiguousarray(
        ST.transpose(0, 2, 7, 5, 1, 4, 3, 6, 8)  # [c,p,q,kq,t,m,g,kl,jq]
        .reshape(CORES, 128, T * 3584)).astype(BF16)

    def pad_vec(v):
        out = np.zeros((NP, 16), np.float32)
        out[:N] = np.asarray(v, np.float32).reshape(N, 16)
        return out.reshape(CORES, T, 128, S, 16)

    X = pad_vec(np.asarray(h0, np.float32)[..., 0])
    M0 = pad_vec(np.asarray(msg0_r0, np.float32)[..., 0])
    XM = (np.stack([X, M0], axis=3).reshape(CORES, T, 128, 224)
          .transpose(0, 2, 1, 3).reshape(CORES, 128, T * 224)).astype(BF16)
    AB = np.ascontiguousarray(np.concatenate([XM, AB], axis=2))
    M1 = np.ascontiguousarray(
        pad_vec(np.asarray(msg1_r0, np.float32)[..., 0])
        .reshape(CORES, T, 128, 112)
        .transpose(0, 2, 1, 3).reshape(CORES, 128, T * 112)).astype(BF16)

    wp = np.zeros((2, NP), np.float32)
    wp[:, :N] = np.asarray(w_pred, np.float32)
    W = np.ascontiguousarray(
        wp.reshape(2, CORES, T, 128, S).transpose(1, 3, 0, 2, 4)
        .reshape(CORES, 128, 2 * GP)).astype(BF16)

    in_maps = [
        {"ab": AB[i], "de": DE[i], "m1": M1[i], "w": W[i]}
        for i in range(CORES)
    ]
    res = run_bass_kernel_spmd(nc, in_maps, list(range(CORES)), trace=TRACE)
    _cache["last_res"] = res
    partial = np.zeros(2, np.float64)
    for i in range(CORES):
        partial += res.results[i]["o"].astype(np.float64).sum(axis=0)
    out = (partial + np.asarray(b_pred, np.float64)).astype(np.float32)
    return out.reshape(1, 2)


# revision 20
# speedup vs baseline: 1.3437x; 1.1754x over previous
"""CartesianMACE rank-0 fused kernel for 8 trn2 NeuronCores (v6).

Only the rank-0 path reaches the output (ranks 1/2 of the reference are
dead code), so per node n with 16x16 mats A=cw0[0,n], B=mw0[0,n],
D=cw1[0,n], E=mw1[0,n] and 16-vecs x=h0[n], m0=msg0_r0[n], m1=msg1_r0[n]:

    s[n] = colsum(D) . (A x + B m0) + colsum(E) . m1
    out  = [sum_n s[n] w_pred[0,n], sum_n s[n] w_pred[1,n]] + b_pred

Design (f32 baseline ~98us; v4 ~71us):
  * All streamed data bf16: halves HBM traffic (13.4MB/core) and gives
    DVE tensor_tensor the 2x perf mode (hardware-verified in traces).
  * DVE-ONLY compute. GpSimd and DVE arbitrate an exclusive lock on the
    shared SBUF port pair: a running GpSimd tensor op makes concurrent
    bf16 2x DVE ops 2.5-4x slower (measured), so GpSimd is kept idle.
  * Reductions are bf16 pairwise fold-trees (2x mode), not 1x
    reduce_sum. A-side products and D/E tiles fold into one shared t8
    tile so deeper levels cover both trees in single fat instructions.
  * The last fold level (stride-2 operands -> 1x mode) is skipped:
    pairs survive into the epilogue, where the cd*t dot product runs
    at doubled width in 2x mode instead (net win).
  * de lands in its own per-span tiles - sharing a tile between DMA
    writes and the mul's engine writes creates a false WAW dependency
    that stalls the ramp (cost ~4us in v4).
  * Nodes on SBUF partitions: 50000 padded to 50176 = 8 cores x 7
    supertiles x 128 partitions x 7 groups. Spans [1,2,2,2] supertiles;
    all DMAs HWDGE with one merged transfer per span half (ab layout
    [P, T*3584] so any slice is one descriptor per partition), ab0+xm
    first for a fast ramp, epilogue-only m1/w last. Per-core (128,2)
    partials are summed on host with b_pred (the head's all-reduce).
"""

import sys

for _p in ("/opt/trn_rl_repo", "/root/.axon_site/_ro/trn_rl_repo"):
    if _p not in sys.path:
        sys.path.append(_p)

import numpy as np
import ml_dtypes

BF16 = ml_dtypes.bfloat16

N, CH = 50000, 16
CORES = 8
T, S = 7, 7          # supertiles per core, groups per supertile
GP = T * S           # 49 groups of 128 nodes per core
NP = CORES * T * 128 * S  # 50176 padded nodes
SPANS = [(0, 1), (1, 2), (3, 2), (5, 2)]  # (first supertile, length)

_cache = {}
TRACE = False


def _split_multiwait(nc, mybir):
    """This walrus build accepts a single sync-wait per instruction, but Tile
    attaches one wait per producer proc. Split: keep the last wait on the
    instruction and hoist the rest onto fresh same-engine Drain carriers
    inserted immediately before it (engines execute their stream in-order,
    so semantics are identical)."""
    for fn in nc.m.functions:
        for bb in fn.blocks:
            insts = bb.instructions  # live list
            i = 0
            while i < len(insts):
                ins = insts[i]
                si = ins.sync_info
                if si is not None and len(si.on_wait) > 1:
                    waits = list(si.on_wait)
                    ins.sync_info = mybir.SyncInfo(
                        on_wait=waits[-1:], on_update=list(si.on_update))
                    for k, w in enumerate(waits[:-1]):
                        insts.insert(i + k, mybir.InstDrain(
                            name=f"{ins.name}_w{k}", opcode="Drain",
                            engine=ins.engine, ins=[], outs=[],
                            sync_info=mybir.SyncInfo(on_wait=[w], on_update=[]),
                        ))
                    i += len(waits) - 1
                i += 1


def _build_nc():
    import concourse.bass as bass
    import concourse.tile as tile
    import concourse.mybir as mybir

    f32 = mybir.dt.float32
    b16 = mybir.dt.bfloat16
    P = 128

    nc = bass.Bass("TRN2", target_bir_lowering=False, debug=False,
                   num_devices=CORES)

    # xm rides at the head of the ab tensor: one transfer (xm+ab span0)
    # gates the first mul instead of two serialized completions
    ab_d = nc.dram_tensor("ab", [P, T * 224 + T * 3584], b16,
                          kind="ExternalInput").ap()
    de_d = nc.dram_tensor("de", [P, T * 3584], b16, kind="ExternalInput").ap()
    m1_d = nc.dram_tensor("m1", [P, T * 112], b16, kind="ExternalInput").ap()
    w_d = nc.dram_tensor("w", [P, 2 * GP], b16, kind="ExternalInput").ap()
    o_d = nc.dram_tensor("o", [P, 2], f32, kind="ExternalOutput").ap()

    F2R = 2 * T * 224  # 3136: [t, m, g, k, r2] per-partition layout
    F1 = T * 224       # 1568

    with tile.TileContext(nc) as tc:
        with (
            tc.tile_pool(name="big", bufs=1) as big,
            tc.tile_pool(name="work", bufs=1) as work,
        ):
            ab_all = big.tile([P, T * 224 + T * 3584], b16)
            xm_sb = ab_all[:, 0:T * 224]
            XO = T * 224            # ab data offset within ab_all
            w_sb = big.tile([P, 2 * GP], b16)
            # ct2[:, 0:3136] = cd pairs (D|E colsums), [:, 3136:] = t pairs
            ct2 = big.tile([P, 2 * F2R], b16)
            vv = big.tile([P, F1], b16)     # [t, sel, g, k]: tn | m1
            cdf = big.tile([P, F1], b16)    # [t, m, g, k]: cd | ce
            pr = big.tile([P, F1], b16)
            tn2 = big.tile([P, F1], b16)    # m-summed t pairs

            # 512B pad staggers tmp/t8 SBUF offsets relative to ab_all:
            # identical TT ops measured 17% faster at this alignment
            # (4660 -> 3891ns for the ts=2 mul) in the v7 layout, which
            # this pad reproduces mod any power-of-2 bank size >= 1024.
            pad = work.tile([P, 256], b16)
            des = []
            for si in range(4):
                de = work.tile([P, 2 * 3584], b16, tag="de", bufs=3)
                des.append(de)
            # stream order: ab leads its consumers by one slot (ab0 as
            # m-halves so the first mul starts one half-transfer earlier,
            # then ab1 before de0, etc.); the DVE stream below is
            # software-pipelined to match (span k's de-fold runs after
            # span k+1's mul).  Epilogue-only m1/w last.
            nc.sync.dma_start(out=ab_all[:, 0:XO + 1792],
                              in_=ab_d[:, 0:XO + 1792])
            nc.sync.dma_start(out=ab_all[:, XO + 1792:XO + 3584],
                              in_=ab_d[:, XO + 1792:XO + 3584])
            nc.sync.dma_start(out=ab_all[:, XO + 3584:XO + 3 * 3584],
                              in_=ab_d[:, XO + 3584:XO + 3 * 3584])
            nc.sync.dma_start(out=des[0][:, 0:3584], in_=de_d[:, 0:3584])
            nc.sync.dma_start(out=ab_all[:, XO + 3 * 3584:XO + 5 * 3584],
                              in_=ab_d[:, XO + 3 * 3584:XO + 5 * 3584])
            nc.sync.dma_start(out=des[1][:, 0:2 * 3584],
                              in_=de_d[:, 3584:3 * 3584])
            nc.sync.dma_start(out=ab_all[:, XO + 5 * 3584:XO + 7 * 3584],
                              in_=ab_d[:, XO + 5 * 3584:XO + 7 * 3584])
            nc.sync.dma_start(out=des[2][:, 0:2 * 3584],
                              in_=de_d[:, 3 * 3584:5 * 3584])
            nc.sync.dma_start(out=des[3][:, 0:2 * 3584],
                              in_=de_d[:, 5 * 3584:7 * 3584])
            # epilogue-only inputs last: m1 into its vv slot (sel=1), w
            nc.sync.dma_start(
                out=vv[:, :].rearrange("p (t s x) -> p t s x",
                                       t=T, s=2, x=112)[:, :, 1],
                in_=m1_d.rearrange("p (t x) -> p t x", t=T, x=112))
            nc.sync.dma_start(out=w_sb[:, :], in_=w_d)

            t4 = work.tile([P, 2 * 1792], b16)
            spans_ctx = []
            for si, (t0, ts) in enumerate(SPANS):
                tmp = work.tile([P, 2 * 3584], b16, tag="tmp", bufs=2)
                t8 = work.tile([P, 2 * 3584], b16, tag="t8", bufs=2)
                spans_ctx.append((t0, ts, tmp, t8))

            gjk = lambda ap, u: ap.rearrange("p (u j k) -> p u j k",
                                             u=u, j=16, k=16)

            def mul_op(si):
                t0, ts, tmp, t8 = spans_ctx[si]
                EN, U = ts * 3584, ts * 14
                if si == 0:     # m-halves: first mul starts half a
                    for h in range(2):  # transfer earlier
                        xm_h = (xm_sb[:, h * 112:(h + 1) * 112]
                                .rearrange("p (u k) -> p u k", u=7, k=16)
                                .unsqueeze(2).broadcast_to((P, 7, 16, 16)))
                        nc.vector.tensor_mul(
                            out=gjk(tmp[:, h * 1792:(h + 1) * 1792], 7),
                            in0=gjk(ab_all[:, XO + h * 1792:
                                           XO + (h + 1) * 1792], 7),
                            in1=xm_h)
                else:
                    xm_bc = (xm_sb[:, t0 * 224:(t0 + ts) * 224]
                             .rearrange("p (u k) -> p u k", u=U, k=16)
                             .unsqueeze(2).broadcast_to((P, U, 16, 16)))
                    nc.vector.tensor_mul(out=gjk(tmp[:, 0:EN], U),
                                         in0=gjk(ab_all[:, XO + t0 * 3584:
                                                        XO + (t0 + ts) * 3584],
                                                 U),
                                         in1=xm_bc)

            def e8a_op(si):
                t0, ts, tmp, t8 = spans_ctx[si]
                EN, HV = ts * 3584, ts * 224
                a16 = tmp[:, 0:EN].rearrange("p (v k) -> p v k", v=HV, k=16)
                e8a = t8[:, HV * 8:HV * 16].rearrange("p (v k) -> p v k",
                                                      v=HV, k=8)
                nc.vector.tensor_add(out=e8a, in0=a16[:, :, 0:8],
                                     in1=a16[:, :, 8:16])

            def tail_op(si):
                # de fold 16->8 + merged [de|ab] 8->4 and 4->2 (pairs)
                t0, ts, tmp, t8 = spans_ctx[si]
                EN, HV, V = ts * 3584, ts * 224, ts * 448
                d16 = des[si][:, 0:EN].rearrange("p (v k) -> p v k",
                                                 v=HV, k=16)
                e8d = t8[:, 0:HV * 8].rearrange("p (v k) -> p v k",
                                                v=HV, k=8)
                nc.vector.tensor_add(out=e8d, in0=d16[:, :, 0:8],
                                     in1=d16[:, :, 8:16])
                e8 = t8[:, 0:V * 8].rearrange("p (v k) -> p v k", v=V, k=8)
                e4 = t4[:, 0:V * 4].rearrange("p (v k) -> p v k", v=V, k=4)
                nc.vector.tensor_add(out=e4, in0=e8[:, :, 0:4],
                                     in1=e8[:, :, 4:8])
                ct_v = (ct2[:, :].rearrange("p (c f) -> p c f", c=2, f=F2R)
                        [:, :, t0 * 448:(t0 + ts) * 448]
                        .rearrange("p c (y r) -> p c y r",
                                   y=ts * 224, r=2))
                nc.vector.tensor_add(
                    out=ct_v,
                    in0=e4[:, :, 0:2].rearrange("p (c y) r -> p c y r",
                                                c=2, y=ts * 224),
                    in1=e4[:, :, 2:4].rearrange("p (c y) r -> p c y r",
                                                c=2, y=ts * 224))

            # software pipeline: span k's de-side tail runs after span
            # k+1's mul, matching the ab-leading DMA stream order above
            mul_op(0); e8a_op(0)
            mul_op(1); tail_op(0); e8a_op(1)
            mul_op(2); tail_op(1); e8a_op(2)
            mul_op(3); tail_op(2); e8a_op(3)
            tail_op(3)

            # ---- epilogue ----
            cd2 = ct2[:, 0:F2R]                  # [t, m, g, k, r] pairs
            t12 = ct2[:, F2R:2 * F2R]
            tmx = lambda ap: ap.rearrange("p (t m x) -> p t m x",
                                          t=T, m=2, x=224)
            # tn2[t,g,k,r] = t12[t,0,..] + t12[t,1,..]   (m-sum, 2x)
            tn2h = tn2[:, 0:F1].rearrange("p (t x) -> p t x", t=T, x=224)
            nc.vector.tensor_add(out=tn2h, in0=tmx(t12)[:, :, 0],
                                 in1=tmx(t12)[:, :, 1])
            # collapse pairs (1x, small): tn -> vv[sel=0]; cd2 -> cdf
            tr = tn2[:, 0:F1].rearrange("p (v r) -> p v r", v=F1 // 2, r=2)
            nc.vector.tensor_add(
                out=vv[:, :].rearrange("p (t s x) -> p t s x",
                                       t=T, s=2, x=112)[:, :, 0],
                in0=tr[:, :, 0].rearrange("p (t x) -> p t x", t=T, x=112),
                in1=tr[:, :, 1].rearrange("p (t x) -> p t x", t=T, x=112))
            cr = cd2.rearrange("p (v r) -> p v r", v=F2R // 2, r=2)
            nc.vector.tensor_add(out=cdf[:, :], in0=cr[:, :, 0],
                                 in1=cr[:, :, 1])
            # pr[t,m,g,k] = cdf * (tn | m1)
            nc.vector.tensor_mul(out=pr[:, :], in0=cdf[:, :], in1=vv[:, :])
            # fold [98, 16] -> [98]
            p16 = pr[:, :].rearrange("p (v k) -> p v k", v=98, k=16)
            h8 = tn2[:, 0:784].rearrange("p (v k) -> p v k", v=98, k=8)
            nc.vector.tensor_add(out=h8, in0=p16[:, :, 0:8],
                                 in1=p16[:, :, 8:16])
            h4 = tn2[:, 784:1176].rearrange("p (v k) -> p v k", v=98, k=4)
            nc.vector.tensor_add(out=h4, in0=h8[:, :, 0:4], in1=h8[:, :, 4:8])
            h2 = tn2[:, 1176:1372].rearrange("p (v k) -> p v k", v=98, k=2)
            nc.vector.tensor_add(out=h2, in0=h4[:, :, 0:2], in1=h4[:, :, 2:4])
            h1 = tn2[:, 1372:1470]
            nc.vector.tensor_add(out=h1, in0=h2[:, :, 0], in1=h2[:, :, 1])
            # s[t,g] = h1[t,0,g] + h1[t,1,g]
            s_all = tn2[:, 1470:1519]
            gm = h1.rearrange("p (t m g) -> p t m g", t=T, m=2, g=S)
            nc.vector.tensor_add(out=s_all.rearrange("p (t g) -> p t g",
                                                     t=T, g=S),
                                 in0=gm[:, :, 0], in1=gm[:, :, 1])
            # head: hm[c, tg] = s[tg] * w[c, tg]; o[c] = sum_tg hm
            hm = pr[:, 0:2 * GP].rearrange("p (c q) -> p c q", c=2, q=GP)
            nc.vector.tensor_mul(
                out=hm,
                in0=w_sb[:, :].rearrange("p (c q) -> p c q", c=2, q=GP),
                in1=s_all.unsqueeze(1).broadcast_to((P, 2, GP)))
            o_sb = big.tile([P, 2], f32)
            nc.vector.reduce_sum(out=o_sb[:, :].rearrange("p c -> p c"),
                                 in_=hm, axis=mybir.AxisListType.X)
            nc.sync.dma_start(out=o_d, in_=o_sb[:, :])

    return nc


def _get_nc():
    if "nc" not in _cache:
        _cache["nc"] = _build_nc()
        import concourse.mybir as mybir
        _split_multiwait(_cache["nc"], mybir)
    return _cache["nc"]


def kernel(h0, cw0, mw0, cw1, mw1,
           msg0_r0, msg0_r1, msg0_r2,
           msg1_r0, msg1_r1, msg1_r2,
           w_pred, b_pred):
    from concourse.bass_utils import run_bass_kernel_spmd

    nc = _get_nc()

    def pad_mat(m):
        out = np.zeros((NP, 256), np.float32)
        out[:N] = np.asarray(m, np.float32).reshape(N, 256)
        return out.reshape(CORES, T, 128, S, 16, 16)  # [c,t,p,g,j,k]

    A5 = pad_mat(cw0[0])
    B5 = pad_mat(mw0[0])
    # AB: [c,t,p, m,g,j,k] -> (c,t,p,3584)
    AB = np.ascontiguousarray(
        np.stack([A5, B5], axis=3).reshape(CORES, T, 128, 3584)
        .transpose(0, 2, 1, 3).reshape(CORES, 128, T * 3584)).astype(BF16)

    D5 = pad_mat(cw1[0])
    E5 = pad_mat(mw1[0])
    # DE: j innermost for the fold tree: [c,t,p, m,g,k,j] -> (c,t,p,3584)
    DE = np.ascontiguousarray(
        np.stack([D5.transpose(0, 1, 2, 3, 5, 4),
                  E5.transpose(0, 1, 2, 3, 5, 4)], axis=3)
        .reshape(CORES, T, 128, 3584)
        .transpose(0, 2, 1, 3).reshape(CORES, 128, T * 3584)).astype(BF16)

    def pad_vec(v):
        out = np.zeros((NP, 16), np.float32)
        out[:N] = np.asarray(v, np.float32).reshape(N, 16)
        return out.reshape(CORES, T, 128, S, 16)

    X = pad_vec(np.asarray(h0, np.float32)[..., 0])
    M0 = pad_vec(np.asarray(msg0_r0, np.float32)[..., 0])
    XM = (np.stack([X, M0], axis=3).reshape(CORES, T, 128, 224)
          .transpose(0, 2, 1, 3).reshape(CORES, 128, T * 224)).astype(BF16)
    AB = np.ascontiguousarray(np.concatenate([XM, AB], axis=2))
    M1 = np.ascontiguousarray(
        pad_vec(np.asarray(msg1_r0, np.float32)[..., 0])
        .reshape(CORES, T, 128, 112)
        .transpose(0, 2, 1, 3).reshape(CORES, 128, T * 112)).astype(BF16)

    wp = np.zeros((2, NP), np.float32)
    wp[:, :N] = np.asarray(w_pred, np.float32)
    W = np.ascontiguousarray(
        wp.reshape(2, CORES, T, 128, S).transpose(1, 3, 0, 2, 4)
        .reshape(CORES, 128, 2 * GP)).astype(BF16)

    in_maps = [
        {"ab": AB[i], "de": DE[i], "m1": M1[i], "w": W[i]}
        for i in range(CORES)
    ]
    res = run_bass_kernel_spmd(nc, in_maps, list(range(CORES)), trace=TRACE)
    _cache["last_res"] = res
    partial = np.zeros(2, np.float64)
    for i in range(CORES):
        partial += res.results[i]["o"].astype(np.float64).sum(axis=0)
    out = (partial + np.asarray(b_pred, np.float64)).astype(np.float32)
    return out.reshape(1, 2)

